# revision 17
# baseline (speedup 1.0000x reference)
"""Trainium2 Bass kernel for nn_CoarseModule (sparse_attention).

Per batch element n (one per NeuronCore, data-parallel over N=8):
  sim[q,s] = (qf[:,q] . mf[:,s]) * 10/256          (Q = S = 48*48 = 2304)
  instance_masks  = softmax over q of sim          -> (Q, S)
  query_masks     = softmax over s of sim, transposed -> (S, Q)
  *_masks_d       = 4x4 patch-pooled sums (clamped at 1-1e-6)
  tmp_*_masks_d   = patch-gathered (row-permuted) copies of the masks

Implementation: three matmul passes, no large transposes.
  Pass A (Q-partition layout): rowsum[q] = sum_s exp(sim) via ACT accum_out
     (softmax without max-subtraction -- values are small, exp is safe),
     plus patch-pooled query_masks_d^T via strided free-dim reduction.
  Pass B (S-partition layout): colsum[s] in-tile; emits query_masks rows
     (ET * bcast(1/rowsum)), tmp_query_masks_d (row-permuted DMA), and
     pooled instance_masks_d^T.
  Pass C (Q-partition layout): emits instance_masks rows
     (E * bcast(1/colsum)) and tmp_instance_masks_d.
  Tail: PE-transpose of the small pooled matrices (2304x144 -> 144x2304)
     (scheduled into the B/C windows).

Each tile runs two 1152-wide PSUM segments (matmul chunks 512/512/128 stay
inside 512-float PSUM banks -- a matmul output must never cross a bank
boundary). The normalize multiply is split DVE/GPSIMD. Big outputs ride the
SP HWDGE ring; permuted tmp outputs ride the GPSIMD SWDGE ring.

Matmuls run in float32r (inputs pre-rounded on DVE): 4x faster than fp32,
~5e-4 relative error on softmax values. Set _USE_F32R = False for exact
fp32 matmuls if tighter accuracy is ever needed.

Cost-model timeline (per core): ~323 us, ~= 48 us stats pass + 258 us
DMA-roofline (92.9 MB at 360 GB/s) + edges.
"""
import numpy as np

N_BATCH = 8
C = 256            # channels (contraction dim)
HW = 48            # H = W
Q = HW * HW        # 2304 spatial positions
PATCH = 4
NG = HW // PATCH   # 12 patches per axis
NP_ = NG * NG      # 144 patches
T = Q // 128       # 18 row tiles
HALF = Q // 2      # 1152
SCALE = 10.0 / C
EPS_CLAMP = 1.0 - 1e-6

_USE_F32R = True
_DEBUG = False

_cache = {}


def _ygroups(t):
    """Split rows q = t*128 + p (p in [0,128)) into same-y groups.

    Returns (p_start, count, dest_base_row) per group, where dest rows in the
    patch-permuted output are R = dest_base_row + (i//4)*16 + i%4 for the
    group's i-th row. Group x-boundaries are always multiples of 4 because
    gcd-structure of 128 and 48 keeps offsets in {0, 16, 32}.
    """
    groups = []
    q0, q1 = t * 128, t * 128 + 128
    while q0 < q1:
        y = q0 // HW
        qe = min(q1, (y + 1) * HW)
        xs = q0 - y * HW
        assert xs % 4 == 0 and (qe - q0) % 4 == 0
        r0 = (y // PATCH) * (NG * 16) + (y % PATCH) * PATCH + (xs // 4) * 16
        groups.append((q0 - t * 128, qe - q0, r0))
        q0 = qe
    return groups


def _build():
    import concourse.bass as bass
    import concourse.mybir as mybir
    import concourse.tile as tile
    from concourse import bacc
    from concourse.masks import make_identity

    F32 = mybir.dt.float32
    F32R = mybir.dt.float32r
    AF = mybir.ActivationFunctionType
    ALU = mybir.AluOpType
    AX = mybir.AxisListType

    nc = bacc.Bacc("TRN2", target_bir_lowering=False, debug=False)

    mf_d = nc.dram_tensor("mask_feats", [C, Q], F32, kind="ExternalInput")
    qf_d = nc.dram_tensor("query_feats", [C, Q], F32, kind="ExternalInput")
    im_o = nc.dram_tensor("instance_masks", [Q, Q], F32, kind="ExternalOutput")
    qm_o = nc.dram_tensor("query_masks", [Q, Q], F32, kind="ExternalOutput")
    imd_o = nc.dram_tensor("instance_masks_d", [NP_, Q], F32, kind="ExternalOutput")
    qmd_o = nc.dram_tensor("query_masks_d", [NP_, Q], F32, kind="ExternalOutput")
    tim_o = nc.dram_tensor("tmp_instance_masks_d", [Q, Q], F32, kind="ExternalOutput")
    tqm_o = nc.dram_tensor("tmp_query_masks_d", [Q, Q], F32, kind="ExternalOutput")
    if _DEBUG:
        dbg = {name: nc.dram_tensor(name, shape, F32, kind="ExternalOutput")
               for name, shape in [
                   ("dbg_rowsum_r", [128, T]), ("dbg_colsum_r", [128, T]),
                   ("dbg_rbc", [128, Q]), ("dbg_cbc", [128, Q]),
                   ("dbg_e_a0", [128, Q]), ("dbg_e_b0", [128, Q]),
                   ("dbg_e_c0", [128, Q]), ("dbg_qf", [128, 2 * Q]),
               ]}

    feat_dt = F32R if _USE_F32R else F32

    with tile.TileContext(nc) as tc:
        with (
            tc.tile_pool(name="feats", bufs=1) as feats,
            tc.tile_pool(name="consts", bufs=1) as consts,
            tc.tile_pool(name="epool", bufs=4) as epool,
            tc.tile_pool(name="opool", bufs=6) as opool,
            tc.tile_pool(name="bcast", bufs=1) as bcast,
            tc.tile_pool(name="pooled", bufs=1) as pooled,
            tc.tile_pool(name="mid", bufs=2) as mid,
            tc.tile_pool(name="small", bufs=6) as small,
            tc.tile_pool(name="flatp", bufs=1) as flatp,
            tc.tile_pool(name="psmm", bufs=2, space="PSUM") as psmm,
            tc.tile_pool(name="pstr", bufs=2, space="PSUM") as pstr,
        ):
            # ---- load features (round to f32r on DVE if enabled) ----
            qf_sb = feats.tile([128, 2, Q], feat_dt, tag="qf")
            mf_sb = feats.tile([128, 2, Q], feat_dt, tag="mf")
            for dst, src in ((qf_sb, qf_d), (mf_sb, mf_d)):
                for k in range(2):
                    for c0 in (0, HALF):
                        raw = epool.tile([128, HALF], F32, tag="eraw")
                        nc.sync.dma_start(
                            raw[:], src[k * 128:(k + 1) * 128, c0:c0 + HALF])
                        nc.vector.tensor_copy(dst[:, k, c0:c0 + HALF], raw[:])

            identity = consts.tile([128, 128], F32)
            make_identity(nc, identity[:])

            rowsum_r = consts.tile([128, T], F32)   # 1/rowsum, q = t*128 + p
            colsum_r = consts.tile([128, T], F32)   # 1/colsum, s = t*128 + p
            flat_r = flatp.tile([1, Q], F32, tag="fr")  # rowsum_r, partition->free
            flat_c = flatp.tile([1, Q], F32, tag="fc")  # colsum_r, partition->free
            qmdT_sb = pooled.tile([128, T, NP_], F32)  # query_masks_d^T
            imdT_sb = pooled.tile([128, T, NP_], F32)  # instance_masks_d^T

            def mm_exp_tile(lhs_sb, rhs_sb, t, recip_out):
                """sim tile [128, Q] -> E = exp(sim*SCALE); optional 1/rowsum.

                PSUM segments are bank-aligned: matmul outputs must not cross
                a 512-float PSUM bank boundary. Q = 2304 = 1024 + 1024 + 256.
                """
                e_t = epool.tile([128, Q], F32, tag="e")
                accs = []
                # two 1152-wide segments; in-seg matmul chunks (512, 512, 128)
                # are bank-aligned (PSUM banks hold 512 floats)
                for h in range(2):
                    g0 = h * HALF
                    ps = psmm.tile([128, HALF], F32, tag="mm")
                    for c0, cw in ((0, 512), (512, 512), (1024, 128)):
                        for k in range(2):
                            nc.tensor.matmul(
                                ps[:, c0:c0 + cw],
                                lhs_sb[:, k, t * 128:(t + 1) * 128],
                                rhs_sb[:, k, g0 + c0:g0 + c0 + cw],
                                start=(k == 0), stop=(k == 1),
                            )
                    if recip_out is not None:
                        acc = small.tile([128, 1], F32, tag="acc")
                        nc.scalar.activation(e_t[:, g0:g0 + HALF], ps[:],
                                             AF.Exp, scale=SCALE, accum_out=acc)
                        accs.append(acc)
                    else:
                        nc.scalar.activation(e_t[:, g0:g0 + HALF], ps[:],
                                             AF.Exp, scale=SCALE)
                if recip_out is not None:
                    ssum = small.tile([128, 1], F32, tag="ssum")
                    nc.vector.tensor_add(ssum[:], accs[0][:], accs[1][:])
                    nc.vector.reciprocal(recip_out, ssum[:])
                return e_t

            def pool_patches(e_t, dst, scalar_ap, halves=(0, 1)):
                """dst half h of 144 = min(patch_sums(e_t) * scalar, EPS)."""
                v = e_t.rearrange("p (gy dy gx dx) -> p gy gx dy dx",
                                  gy=NG, dy=PATCH, gx=NG, dx=PATCH)
                nh = NP_ // 2
                for h in halves:
                    red = mid.tile([128, nh], F32, tag="red")
                    nc.vector.reduce_sum(
                        red[:], v[:, h * (NG // 2):(h + 1) * (NG // 2)],
                        axis=AX.XY)
                    nc.vector.tensor_scalar(
                        out=dst[:, h * nh:(h + 1) * nh], in0=red[:],
                        scalar1=scalar_ap, scalar2=EPS_CLAMP,
                        op0=ALU.mult, op1=ALU.min)

            def emit_rows(e_t, bc_sb, t, out_dram, tmp_dram):
                """masks rows = e_t * bcast; DMA natural + patch-permuted."""
                o_t = opool.tile([128, Q], F32, tag="o")
                nc.vector.tensor_mul(o_t[:, :1536], e_t[:, :1536],
                                     bc_sb[:, :1536])
                nc.gpsimd.tensor_mul(o_t[:, 1536:], e_t[:, 1536:],
                                     bc_sb[:, 1536:])
                nc.sync.dma_start(out_dram[t * 128:(t + 1) * 128, :], o_t[:])
                base = tmp_dram[:, :]
                for pa, cnt, r0 in _ygroups(t):
                    dst = bass.AP(tensor=base.tensor, offset=r0 * Q,
                                  ap=[[16 * Q, cnt // 4], [PATCH * Q // PATCH, PATCH],
                                      [1, Q]])
                    nc.gpsimd.dma_start(dst, o_t[pa:pa + cnt, :])

            def make_bcast(flat):
                """flat [1, Q] of reciprocals -> [128, Q] broadcast tile."""
                bc = bcast.tile([128, Q], F32)
                nc.gpsimd.partition_broadcast(bc[:, :HALF], flat[:1, :HALF])
                nc.gpsimd.partition_broadcast(bc[:, HALF:], flat[:1, HALF:])
                return bc

            def pooled_tail(src_sb, out_dram, h0s=(0, 72)):
                tgroups = [(0, 4), (4, 4), (8, 4), (12, 4), (16, 2)]
                for h0 in h0s:
                    ro = opool.tile([128, Q], F32, tag="o")
                    for g0, gn in tgroups:
                        pt = pstr.tile([72, 512], F32, tag="tr")
                        for j in range(gn):
                            nc.tensor.transpose(
                                pt[:, j * 128:(j + 1) * 128],
                                src_sb[:, g0 + j, h0:h0 + 72],
                                identity[:])
                        nc.scalar.copy(ro[:72, g0 * 128:(g0 + gn) * 128],
                                       pt[:, :gn * 128])
                    nc.sync.dma_start(out_dram[h0:h0 + 72, :], ro[:72, :])

            # ---- Pass A: Q-layout; rowsum + pooled query_masks_d^T ----
            for t in range(T):
                rr = rowsum_r[:, t:t + 1]
                e_t = mm_exp_tile(qf_sb, mf_sb, t, rr)
                pool_patches(e_t, qmdT_sb[:, t, :], rr, halves=(0,))
                nc.sync.dma_start(flat_r[:1, t * 128:(t + 1) * 128], rr)
                if _DEBUG and t == 0:
                    nc.sync.dma_start(dbg["dbg_e_a0"][:, :], e_t[:])

            rbc = make_bcast(flat_r)
            pooled_tail(qmdT_sb, qmd_o, h0s=(0,))
            if _DEBUG:
                nc.sync.dma_start(dbg["dbg_rowsum_r"][:, :], rowsum_r[:])
                nc.sync.dma_start(dbg["dbg_rbc"][:, :], rbc[:])
                qfv = qf_sb.bitcast(F32) if _USE_F32R else qf_sb
                nc.sync.dma_start(dbg["dbg_qf"][:, :],
                                  qfv[:].rearrange("p k q -> p (k q)"))

            # ---- Pass B: S-layout; colsum, query_masks, tmp_qm_d, im_d^T ----
            for t in range(T):
                cr = colsum_r[:, t:t + 1]
                e_t = mm_exp_tile(mf_sb, qf_sb, t, cr)
                pool_patches(e_t, imdT_sb[:, t, :], cr)
                nc.scalar.dma_start(flat_c[:1, t * 128:(t + 1) * 128], cr)
                emit_rows(e_t, rbc, t, qm_o, tqm_o)
                if _DEBUG and t == 0:
                    nc.sync.dma_start(dbg["dbg_e_b0"][:, :], e_t[:])

            cbc = make_bcast(flat_c)
            pooled_tail(imdT_sb, imd_o)

            if _DEBUG:
                nc.sync.dma_start(dbg["dbg_colsum_r"][:, :], colsum_r[:])
                nc.sync.dma_start(dbg["dbg_cbc"][:, :], cbc[:])

            # ---- Pass C: Q-layout; instance_masks + tmp_im_d ----
            for t in range(T):
                e_t = mm_exp_tile(qf_sb, mf_sb, t, None)
                pool_patches(e_t, qmdT_sb[:, t, :], rowsum_r[:, t:t + 1],
                             halves=(1,))
                emit_rows(e_t, cbc, t, im_o, tim_o)
                if _DEBUG and t == 0:
                    nc.sync.dma_start(dbg["dbg_e_c0"][:, :], e_t[:])

            pooled_tail(qmdT_sb, qmd_o, h0s=(72,))


    nc.compile()
    return nc


def _get_nc():
    if "nc" not in _cache:
        _cache["nc"] = _build()
    return _cache["nc"]


def kernel(mask_feats, query_feats_all, path_size):
    assert int(path_size) == PATCH
    mask_feats = np.ascontiguousarray(mask_feats, dtype=np.float32)
    query_feats_all = np.ascontiguousarray(query_feats_all, dtype=np.float32)
    n = mask_feats.shape[0]
    assert n == N_BATCH and mask_feats.shape == (n, C, HW, HW)

    from concourse.bass_utils import run_bass_kernel_spmd

    nc = _get_nc()
    in_maps = [
        {
            "mask_feats": mask_feats[i].reshape(C, Q),
            "query_feats": query_feats_all[i].reshape(C, Q),
        }
        for i in range(n)
    ]
    res = run_bass_kernel_spmd(nc, in_maps, core_ids=list(range(n))).results

    im = np.stack([r["instance_masks"] for r in res])
    qm = np.stack([r["query_masks"] for r in res])
    imd = np.stack([r["instance_masks_d"] for r in res])
    qmd = np.stack([r["query_masks_d"] for r in res])
    tim = np.stack([r["tmp_instance_masks_d"] for r in res]).reshape(
        n, NP_, PATCH * PATCH, Q)
    tqm = np.stack([r["tmp_query_masks_d"] for r in res]).reshape(
        n, NP_, PATCH * PATCH, Q)
    return im, qm, imd, qmd, tim, tqm
